# revision 59
# baseline (speedup 1.0000x reference)
"""Trainium2 Bass kernel for nn_AttnFree_Block (AFT + neural adaptive pairwise bias).

Sharding: 8 cores over the (B=2, T=512) query-row grid -> 128 query rows/core.
Each core computes the full pairwise bias network for its 128 rows x 512 keys,
then the AFT attention + FFN for its rows. Output rows are gathered on host.

v3 restructure vs v2 (332us -> 270us):
- stats matmuls col-tiled 4-way: each chunk's 8 stats accumulate in its own
  32-partition PSUM band via tile_position=(0,32c); the four 128x32 PE tiles
  run concurrently, cutting stats matmul time ~4x.
- gate-hidden matmuls col-tiled 2-way (two concurrent 128x64 tiles/pair).
- h matmuls row-tiled (32-strip weights) run concurrently per pair.
- e matmul + evac done pair-wide ([128,1024] tiles, one wide cast); the e/h
  PSUM tiles share one 2-buf rotating pool (h0,e0,h1,e1 ping-pong).
- qc = e0*e1 in one wide [64,1024] mul into static base-0 tiles (rows 64:128
  zeroed once) so a single stats weight block serves both chunk parities.
- scalar FIFO spine: sin(q+1) emitted right after silu(q,0) so the
  sin->h->silu cycle never waits behind gl-silu/evac work.
- gate sigmoid linearized (|x|<=0.06 -> 0.5+x/4, 5e-6 abs) -> no table swap.
- stats transpose-evac via bf16 selector matmul; inputs packed into 3 wide
  DMAs instead of ~40 small ones.
- softmax(adapt_bias) computed in transposed [j, i] layout -> no A transposes.
"""
import sys
sys.path.insert(0, '/opt/trn_rl_repo')

import math
import numpy as np
import ml_dtypes

BF = ml_dtypes.bfloat16

import concourse.bass as bass
import concourse.bacc as bacc
import concourse.mybir as mybir
import concourse.tile as tile
from concourse import bass_utils

F32 = mybir.dt.float32
F32R = mybir.dt.float32r
BF16 = mybir.dt.bfloat16

AF = mybir.ActivationFunctionType
ALU = mybir.AluOpType
AX = mybir.AxisListType

B, T, D, H, HID, FF = 2, 512, 128, 128, 64, 512
NCORES = 8
RPC = T * B // NCORES  # 128 query rows per core
NQ = RPC // 4          # 32 quads
EPS_LN = 1e-5
EPS_RMS = 1e-5
EPSX = 1e-4            # epsilon for the sin(eps*x)/eps ~= x identity-row trick
PI = math.pi

_CACHE = {}


def _build_program():
    nc = bacc.Bacc()

    def din(name, shape, dt=F32):
        return nc.dram_tensor(name, list(shape), dt, kind="ExternalInput")

    # consolidated input packs (few big DMAs instead of ~40 small ones)
    t_pack2 = din("pack2", (2, 1280))          # clx|cly|crx|cry
    t_packF = din("packF", (128, 1033))        # f32 constants, see PF offsets
    t_packB = din("packB", (128, 3456), BF16)  # bf16 weights, see PB offsets
    t_selC = din("selC", (128, 4096), BF16)   # per-quad one-hot row selectors (cost)
    t_selA = din("selA", (128, 4096), BF16)   # per-quad one-hot row selectors (angle)

    t_out = nc.dram_tensor("out", [RPC, D], F32, kind="ExternalOutput")
    return nc, locals()


def _emit(nc, tt, consts):
    gb2d = consts["gb2d"]
    gps64 = consts["gps64"]

    with tile.TileContext(nc) as tc:
        import contextlib
        with contextlib.ExitStack() as ctx:
            singles = ctx.enter_context(tc.tile_pool(name="singles", bufs=1))
            sb = ctx.enter_context(tc.tile_pool(name="sb", bufs=1))

            _eng_rr = [nc.gpsimd, nc.scalar, nc.sync]

            def load1(t, shape, pool=None, eng=None):
                p = pool or singles
                s = p.tile(list(shape), t.dtype, tag=t.name + "_sb", name=t.name + "_sb")
                e = eng if eng is not None else _eng_rr[load1._i % 3]
                load1._i += 1
                e.dma_start(out=s[:, :], in_=t.ap()[:, :])
                return s
            load1._i = 0

            # ---- consolidated loads: a handful of wide DMAs ----
            pack2 = singles.tile([2, 1280], F32, tag="pack2_sb", name="pack2_sb")
            nc.sync.dma_start(out=pack2[:, :], in_=tt["t_pack2"].ap()[:, :])
            packB = singles.tile([128, 3456], BF16, tag="packB_sb", name="packB_sb")
            nc.scalar.dma_start(out=packB[:, 0:1728],
                                in_=tt["t_packB"].ap()[:, 0:1728])
            nc.scalar.dma_start(out=packB[:, 1728:3456],
                                in_=tt["t_packB"].ap()[:, 1728:3456])
            packF = singles.tile([128, 1033], F32, tag="packF_sb", name="packF_sb")
            nc.gpsimd.dma_start(out=packF[:, 0:517], in_=tt["t_packF"].ap()[:, 0:517])
            nc.gpsimd.dma_start(out=packF[:, 517:1033],
                                in_=tt["t_packF"].ap()[:, 517:1033])
            selC = singles.tile([128, 4096], BF16, tag="selC_sb", name="selC_sb")
            selA = singles.tile([128, 4096], BF16, tag="selA_sb", name="selA_sb")
            for hh in range(2):
                nc.sync.dma_start(out=selC[:, 2048 * hh:2048 * hh + 2048],
                                  in_=tt["t_selC"].ap()[:, 2048 * hh:2048 * hh + 2048])
                nc.gpsimd.dma_start(out=selA[:, 2048 * hh:2048 * hh + 2048],
                                    in_=tt["t_selA"].ap()[:, 2048 * hh:2048 * hh + 2048])

            clx = pack2[:, 0:RPC]
            cly = pack2[:, RPC:2 * RPC]
            crx = pack2[:, 256:256 + T]
            cry = pack2[:, 768:768 + T]
            sinscale = packF[:, 0:1]
            sinbias = packF[:, 1:2]
            b1s = packF[:, 2:3]
            gb1s = packF[:, 3:4]
            bq = packF[:, 4:5]
            ffb1s = packF[:, 5:9]
            ident = packF[:, 9:137]
            onesB = packF[:, 137:265]
            bvb = packF[:, 265:393]
            xrows = packF[:, 393:521]
            bob = packF[:, 521:649]
            rms1b = packF[:, 649:777]
            rms2b = packF[:, 777:905]
            ffb2b = packF[:, 905:1033]
            xT = packB[:, 0:512]
            xrowsT = packB[:, 512:640]
            cost_sb = packB[:, 640:1152]
            w1big = packB[:, 1152:1280]
            w2x = packB[:, 1280:1408]
            glWE = packB[:, 1408:1536]
            glWO = packB[:, 1536:1664]
            statsW = packB[:, 1664:1856]
            selT = packB[:, 1856:1888]
            onesM = packB[:, 1888:1920]
            wq = packB[:, 1920:2048]
            wk = packB[:, 2048:2176]
            wv = packB[:, 2176:2304]
            wo = packB[:, 2304:2432]
            ffw1 = packB[:, 2432:2944]
            ffw2 = packB[:, 2944:3456]

            # static double-buffered crx tiles: qc products live in partitions
            # 0:64 (cols 0:T = even chunk, T:2T = odd); rows 64:128 are zeroed
            # once so the stats matmul can read all 128 partitions safely.
            mixT = []
            for mb in range(4):
                mt = singles.tile([128, 2 * T], BF16, tag=f"mixT{mb}")
                nc.gpsimd.memset(mt[64:128, :], 0.0)
                mixT.append(mt)

            c_epsln = singles.tile([128, 1], F32, tag="c_epsln")
            nc.vector.memset(c_epsln[:, :], EPS_LN)
            c_epsrms = singles.tile([128, 1], F32, tag="c_epsrms")
            nc.vector.memset(c_epsrms[:, :], EPS_RMS)
            c_gb2d = singles.tile([128, 1], F32, tag="c_gb2d")
            nc.vector.memset(c_gb2d[:, :], gb2d)

            # ================= Phase A: angle matrix =================
            with tc.tile_pool(name="pA", bufs=2, space="PSUM") as pA, \
                 tc.tile_pool(name="paS", bufs=1) as paS:
                dx_ps = pA.tile([RPC, T], F32, tag="dxy")
                nc.tensor.matmul(dx_ps[:, :], clx[:, :], crx[:, :], start=True, stop=True)
                dy_ps = pA.tile([RPC, T], F32, tag="dxy")
                nc.tensor.matmul(dy_ps[:, :], cly[:, :], cry[:, :], start=True, stop=True)
                negx = paS.tile([RPC, T], F32, tag="negx")
                nc.vector.tensor_scalar(negx[:, :], dx_ps[:, :], -1.0, None, op0=ALU.mult)
                dxa = paS.tile([RPC, T], F32, tag="dxa")
                nc.vector.tensor_tensor(out=dxa[:, :], in0=negx[:, :], in1=dx_ps[:, :], op=ALU.max)
                cmpneg = paS.tile([RPC, T], F32, tag="cmpneg")
                nc.vector.tensor_scalar(cmpneg[:, :], dx_ps[:, :], 0.0, None, op0=ALU.is_lt)
                sdy = paS.tile([RPC, T], F32, tag="sdy")
                nc.vector.tensor_scalar(sdy[:, :], dy_ps[:, :], 0.0, None, op0=ALU.is_ge)
                nc.vector.tensor_scalar(sdy[:, :], sdy[:, :], 2.0, -1.0, op0=ALU.mult, op1=ALU.add)
                dya = paS.tile([RPC, T], F32, tag="dya")
                nc.vector.tensor_mul(dya[:, :], sdy[:, :], dy_ps[:, :])
                mx = paS.tile([RPC, T], F32, tag="mx")
                nc.vector.tensor_tensor(out=mx[:, :], in0=dxa[:, :], in1=dya[:, :], op=ALU.max)
                nc.vector.tensor_scalar_max(mx[:, :], mx[:, :], 1e-30)
                mn = paS.tile([RPC, T], F32, tag="mn")
                nc.vector.tensor_tensor(out=mn[:, :], in0=dxa[:, :], in1=dya[:, :], op=ALU.min)
                nc.vector.reciprocal(mx[:, :], mx[:, :])
                rt = paS.tile([RPC, T], F32, tag="rt")
                nc.vector.tensor_mul(rt[:, :], mn[:, :], mx[:, :])
                at = paS.tile([RPC, T], F32, tag="at")
                nc.scalar.activation(at[:, :], rt[:, :], AF.Arctan)
                swap = paS.tile([RPC, T], F32, tag="swap")
                nc.vector.tensor_tensor(out=swap[:, :], in0=dya[:, :], in1=dxa[:, :], op=ALU.is_gt)
                v1 = paS.tile([RPC, T], F32, tag="v1")
                nc.vector.tensor_scalar(v1[:, :], at[:, :], -2.0, PI / 2, op0=ALU.mult, op1=ALU.add)
                nc.gpsimd.tensor_mul(v1[:, :], v1[:, :], swap[:, :])
                base = paS.tile([RPC, T], F32, tag="base")
                nc.gpsimd.tensor_add(base[:, :], at[:, :], v1[:, :])
                v2 = paS.tile([RPC, T], F32, tag="v2")
                nc.vector.tensor_scalar(v2[:, :], base[:, :], -2.0, PI, op0=ALU.mult, op1=ALU.add)
                nc.gpsimd.tensor_mul(v2[:, :], v2[:, :], cmpneg[:, :])
                nc.gpsimd.tensor_add(base[:, :], base[:, :], v2[:, :])
                angle = sb.tile([RPC, T], BF16, tag="angle")
                nc.vector.tensor_mul(angle[:, :], sdy[:, :], base[:, :])

                # ---- Phase D head (independent of bias net): runs in the
                # phase-A window while the PE is otherwise idle ----
                with tc.tile_pool(name="psDH", bufs=2, space="PSUM") as psDH:
                    kl_ps = pA.tile([H, T], F32, tag="dxy", name="kl_ps")
                    nc.tensor.matmul(kl_ps[:, :], wk[:, :], xT[:, :], start=True, stop=True)
                    q_ps = psDH.tile([H, RPC], F32, tag="scrDH", name="q_ps")
                    nc.tensor.matmul(q_ps[:, :], wq[:, :], xrowsT[:, :], start=True, stop=True)
                    qs_sb = sb.tile([H, RPC], F32, tag="qs_sb")
                    nc.scalar.activation(qs_sb[:, :], q_ps[:, :], AF.Sigmoid, bias=bq[:, :])

                    kex = paS.tile([H, T], F32, tag="kex")
                    nc.scalar.activation(kex[:, :], kl_ps[:, :], AF.Exp)
                    ksum = paS.tile([H, 1], F32, tag="ksum")
                    nc.vector.reduce_sum(ksum[:, :], kex[:, :], axis=AX.X)
                    nc.vector.reciprocal(ksum[:, :], ksum[:, :])
                    K_sb = paS.tile([H, T], F32, tag="K_sb")
                    nc.vector.tensor_scalar_mul(K_sb[:, :], kex[:, :], ksum[:, :])
                    eK = paS.tile([H, T], F32, tag="eK")
                    nc.scalar.activation(eK[:, :], K_sb[:, :], AF.Exp)
                    eKT = sb.tile([128, T], BF16, tag="eKT")
                    for tb in range(4):
                        tp = psDH.tile([128, 128], F32, tag="scrDH", name="tp")
                        nc.tensor.transpose(tp[:, :], eK[:, 128 * tb:128 * tb + 128], ident[:, :])
                        nc.vector.tensor_copy(out=eKT[:, 128 * tb:128 * tb + 128], in_=tp[:, :])
                    ekv = sb.tile([128, T], BF16, tag="ekv")
                    for tb in range(4):
                        v_ps = psDH.tile([128, H], F32, tag="scrDH", name="v_ps")
                        nc.tensor.matmul(v_ps[:, :], xT[:, 128 * tb:128 * tb + 128], wv[:, :],
                                         start=True, stop=True)
                        vb = paS.tile([128, H], BF16, tag="vb", name="vb")
                        nc.vector.tensor_add(vb[:, :], v_ps[:, :], bvb[:, :])
                        nc.vector.tensor_mul(ekv[:, 128 * tb:128 * tb + 128], vb[:, :],
                                             eKT[:, 128 * tb:128 * tb + 128])

            # ================= Phase B: bias-net main loop =================
            T_coll = singles.tile([128, 4096], F32, tag="Tcoll")
            # s-major layout: col = s*512 + jb*128 + i  -> S[k] slices are contiguous
            Tout = T_coll[:, :].rearrange("p (s jb i) -> p jb i s", s=8, jb=4, i=128)
            S = [T_coll[:, 512 * k:512 * k + 512] for k in range(8)]

            with tc.tile_pool(name="feS", bufs=2) as feS, \
                 tc.tile_pool(name="hS", bufs=6) as hS, \
                 tc.tile_pool(name="eS", bufs=4) as eS, \
                 tc.tile_pool(name="hzS", bufs=8) as hzS, \
                 tc.tile_pool(name="ggS", bufs=4) as ggS, \
                 tc.tile_pool(name="evS", bufs=2) as evS, \
                 tc.tile_pool(name="psFS", bufs=2, space="PSUM") as psFS, \
                 tc.tile_pool(name="psHE", bufs=2, space="PSUM") as psHE, \
                 tc.tile_pool(name="psGL", bufs=1, space="PSUM") as psGL, \
                 tc.tile_pool(name="psTR", bufs=1, space="PSUM") as psTR:

                state = {}

                def produce_quad(q):
                    # fe for the 4 chunks of quad q
                    fe_ps = psFS.tile([128, T], F32, tag="fs_ps", name="fe_ps")
                    nc.tensor.matmul(fe_ps[:, :], selC[:, 128 * q:128 * q + 128],
                                     cost_sb[:, :], start=True, stop=False)
                    nc.tensor.matmul(fe_ps[:, :], selA[:, 128 * q:128 * q + 128],
                                     angle[:, :], start=False, stop=True)
                    fe_sb = feS.tile([128, T], BF16, tag="fe_sb", name="fe_sb")
                    nc.scalar.activation(fe_sb[:, :], fe_ps[:, :], AF.Sin,
                                         scale=sinscale[:, :], bias=sinbias[:, :])
                    state[("fe", q)] = fe_sb

                def h_mms(q, k):
                    fe_sb = state[("fe", q)]
                    h2_ps = psHE.tile([128, 2 * T], F32, tag="he_ps", name="h2_ps")
                    for half in (0, 1):
                        c = 2 * k + half
                        nc.tensor.matmul(h2_ps[:, T * half:T * half + T],
                                         w1big[32 * c:32 * c + 18, :],
                                         fe_sb[32 * c:32 * c + 18, :], start=True, stop=True,
                                         tile_position=(32 * c, 0))
                    state[("h2ps", q, k)] = h2_ps

                def pair_silu(q, k):
                    h2_ps = state.pop(("h2ps", q, k))
                    h2_sb = hS.tile([128, 2 * T], BF16, tag="h2_sb", name="h2_sb")
                    nc.scalar.activation(h2_sb[:, :], h2_ps[:, :], AF.Silu, bias=b1s[:, :])
                    state[("h2", q, k)] = h2_sb

                def e_pair(q, k):
                    # e = [W0.T h0; W1.T h1] for chunks 2k, 2k+1 into one
                    # [128, 2T] tile; one wide cast halves the V evac cost.
                    h2 = state[("h2", q, k)]
                    e2_ps = psHE.tile([128, 2 * T], F32, tag="he_ps", name="e2_ps")
                    for half in (0, 1):
                        nc.tensor.matmul(e2_ps[:, T * half:T * half + T], w2x[:, :],
                                         h2[:, T * half:T * half + T],
                                         start=True, stop=True)
                    e2_sb = eS.tile([128, 2 * T], BF16, tag="e_sb", name="e2_sb")
                    nc.vector.tensor_copy(out=e2_sb[:, :], in_=e2_ps[:, :])
                    state[("e2ps", q, k)] = e2_ps
                    state[("e2", q, k)] = e2_sb

                def sq_pair(q, k):
                    e2_sb = state[("e2", q, k)]
                    sq_sb = hzS.tile([128, 2 * T], BF16, tag="hz_sb", name="sq_sb")
                    nc.gpsimd.tensor_mul(sq_sb[:, :], e2_sb[:, :], e2_sb[:, :])
                    state[("hz2", q, k)] = sq_sb

                def crx_mul(q, k):
                    # qc products for both chunks of pair k in one wide op:
                    # mixT[0:64, 0:2T] = e0 * e1 (psum x sbuf, partitions 0:64)
                    crx_sb = mixT[(2 * q + k) % 4]
                    e2_ps = state.pop(("e2ps", q, k))
                    e2_sb = state.pop(("e2", q, k))
                    nc.vector.tensor_mul(crx_sb[0:64, :],
                                         e2_ps[0:64, :], e2_sb[64:128, :])
                    state[("mix", q, k)] = crx_sb

                def pair_gl_mms(q, k):
                    # gate-hidden for both chunks of pair k: two concurrent
                    # 128x64 col-tiles (even chunk -> rows 0:64, odd -> 64:128)
                    gl_ps = psGL.tile([128, T], F32, tag="gl_ps", name="gl_ps")
                    h2 = state[("h2", q, k)]
                    nc.tensor.matmul(gl_ps[0:64, :], glWE[:, 0:64], h2[:, 0:T],
                                     start=True, stop=True, tile_position=(0, 0),
                                     skip_group_check=True)
                    nc.tensor.matmul(gl_ps[64:128, :], glWO[:, 64:128], h2[:, T:2 * T],
                                     start=True, stop=True, tile_position=(0, 64),
                                     skip_group_check=True)
                    state[("glps", q, k)] = gl_ps

                def gl_silu(q, k):
                    gl_ps = state.pop(("glps", q, k))
                    gg_sb = ggS.tile([128, T], BF16, tag="gg_sb", name="gg_sb")
                    nc.scalar.activation(gg_sb[:, :], gl_ps[:, :], AF.Silu, bias=gb1s[:, :])
                    state[("gg", q, k)] = gg_sb

                # ---- stats: 4 concurrent 128x32 col-tiles, one per chunk ----
                # band c = psum partitions 32c..32c+32; stats at rows 32c+s.
                # statsW blocks: 0=h, 1=sq, 2=crx-even, 3=crx-odd, 4=gg-even,
                # 5=gg-odd (each [128,32] with stats in cols 0..7).
                def stats_round_h(q):
                    st_ps = psFS.tile([128, T], F32, tag="fs_ps", name="st_ps")
                    for c in range(4):
                        h_sl = state[("h2", q, c // 2)][:, T * (c % 2):T * (c % 2) + T]
                        nc.tensor.matmul(st_ps[32 * c:32 * c + 32, :],
                                         statsW[:, 0:32], h_sl,
                                         start=True, stop=False,
                                         tile_position=(0, 32 * c),
                                         skip_group_check=True)
                    state[("st", q)] = st_ps

                def stats_round_sq(q):
                    st_ps = state[("st", q)]
                    for c in range(4):
                        sq2 = state[("hz2", q, c // 2)]
                        nc.tensor.matmul(st_ps[32 * c:32 * c + 32, :],
                                         statsW[:, 32:64],
                                         sq2[:, T * (c % 2):T * (c % 2) + T],
                                         start=False, stop=False,
                                         tile_position=(0, 32 * c),
                                         skip_group_check=True)
                    for k in range(2):
                        state.pop(("hz2", q, k))

                def stats_round_xg(q):
                    st_ps = state[("st", q)]
                    for k in range(2):
                        mix = state.pop(("mix", q, k))
                        for par in range(2):
                            c = 2 * k + par
                            nc.tensor.matmul(st_ps[32 * c:32 * c + 32, :],
                                             statsW[:, 64:96],
                                             mix[:, T * par:T * par + T],
                                             start=False, stop=False,
                                             tile_position=(0, 32 * c),
                                             skip_group_check=True)
                    for k in range(2):
                        gg = state.pop(("gg", q, k))
                        for par in range(2):
                            c = 2 * k + par
                            nc.tensor.matmul(st_ps[32 * c:32 * c + 32, :],
                                             statsW[:, 128 + 32 * par:160 + 32 * par],
                                             gg[:, :], start=False, stop=True,
                                             tile_position=(0, 32 * c),
                                             skip_group_check=True)
                    for k in range(2):
                        state.pop(("h2", q, k))

                def evac_copy(q):
                    st_ps = state[("st", q)]
                    sts_sb = evS.tile([128, T], BF16, tag="sts_sb", name="sts_sb")
                    nc.scalar.activation(sts_sb[:, :], st_ps[:, :], AF.Copy)
                    state[("sts", q)] = sts_sb

                def evac_transpose(q):
                    state.pop(("st", q))
                    sts_sb = state[("sts", q)]
                    trp_ps = psTR.tile([128, 128], BF16, tag="trp_ps", name="trp_ps")
                    for jb in range(4):
                        # out[j, 8c+s] = sts[32c+s, 128jb+j] via selector matmul
                        nc.tensor.transpose(trp_ps[:, 32 * jb:32 * jb + 32],
                                            sts_sb[:, 128 * jb:128 * jb + 128],
                                            selT[:, :])
                    state[("trp", q)] = trp_ps

                def evac_tcoll(q):
                    trp_ps = state.pop(("trp", q))
                    state.pop(("sts", q))
                    stv = trp_ps[:, :].rearrange("p (jb c s) -> p jb c s", jb=4, c=4)
                    nc.vector.tensor_copy(out=Tout[:, :, 4 * q:4 * q + 4, 0:8],
                                          in_=stv[:, :, :, 0:8])

                # Steady-state spine: both h-MM pairs issue back-to-back, the
                # two h-silus run back-to-back on scalar, and sin(q+1) follows
                # immediately — the scalar FIFO no longer interleaves gl/evac
                # work into the latency-critical sin->h->silu->sin cycle.
                # scalar FIFO per iter: silu0, sin(q+1), gl-silu(q-1,1),
                # silu1, gl-silu(q,0), evac(q-1) -- gl1's silu is deferred one
                # iteration so the S queue never bunches up between the two
                # latency-critical h-silus.
                produce_quad(0)
                for q in range(NQ):
                    h_mms(q, 0)
                    pair_silu(q, 0)
                    if q + 1 < NQ:
                        produce_quad(q + 1)
                    if q >= 1:
                        gl_silu(q - 1, 1)
                        stats_round_h(q - 1)
                        stats_round_sq(q - 1)
                    e_pair(q, 0)
                    sq_pair(q, 0)
                    h_mms(q, 1)
                    pair_silu(q, 1)
                    crx_mul(q, 0)
                    pair_gl_mms(q, 0)
                    gl_silu(q, 0)
                    if q >= 1:
                        stats_round_xg(q - 1)
                        evac_copy(q - 1)
                    e_pair(q, 1)
                    sq_pair(q, 1)
                    crx_mul(q, 1)
                    pair_gl_mms(q, 1)
                    if q >= 1:
                        evac_transpose(q - 1)
                        evac_tcoll(q - 1)
                q = NQ
                gl_silu(q - 1, 1)
                stats_round_h(q - 1)
                stats_round_sq(q - 1)
                stats_round_xg(q - 1)
                evac_copy(q - 1)
                evac_transpose(q - 1)
                evac_tcoll(q - 1)

            # ================= Phase C/D: bias assembly + AFT + FFN =================

            with tc.tile_pool(name="ph2", bufs=1) as ph2, \
                 tc.tile_pool(name="ph2s", bufs=4) as ph2s, \
                 tc.tile_pool(name="phD", bufs=1) as phD, \
                 tc.tile_pool(name="psD", bufs=3, space="PSUM") as psD, \
                 tc.tile_pool(name="psD2", bufs=1, space="PSUM") as psD2:

                # keep the PE clock ramped through the scalar/vector-only
                # phase-C window: idle gaps reset the p-state and the whole
                # attention/FFN tail then runs at the mid clock.  ~8us of
                # dependency-free filler matmuls hold the ramp.
                warm_ps = psD.tile([128, RPC], F32, tag="scrD", name="warm_ps")
                for _w in range(110):
                    nc.tensor.matmul(warm_ps[:, :], w2x[:, :], selC[:, 0:RPC],
                                     start=True, stop=True, skip_group_check=True)

                # ---- Phase C: assemble adaptive bias in [j, i] layout ----
                def dt(tag):
                    return ph2.tile([128, T], F32, tag=tag, name=tag)

                def dscr():
                    return ph2s.tile([128, T], F32, tag="cscr", name="cscr")

                # stats arrive pre-combined: S0=a0-a1, S1=a1, S2=b0-b1, S3=b1,
                # S4=q00+q11-2qc, S5=q11, S6=qc-q11, S7=d
                # gate sigmoid linearized: |x|<=0.06 -> sigmoid(x)=0.5+x/4 (5e-6 abs)
                g0t = dt("g0t")
                nc.vector.tensor_scalar(g0t[:, :], S[7], 0.25, 0.25 * gb2d + 0.5,
                                        op0=ALU.mult, op1=ALU.add)
                s0 = dt("s0"); nc.vector.tensor_mul(s0[:, :], g0t[:, :], S[0])
                nc.vector.tensor_tensor(out=s0[:, :], in0=s0[:, :], in1=S[1], op=ALU.add)
                s1 = dt("s1"); nc.vector.tensor_mul(s1[:, :], g0t[:, :], S[2])
                nc.vector.tensor_tensor(out=s1[:, :], in0=s1[:, :], in1=S[3], op=ALU.add)
                # sum(fused^2) = q11 + g*(2*(qc - q11) + g*(q00 + q11 - 2qc))
                m1 = dscr(); nc.vector.tensor_mul(m1[:, :], g0t[:, :], S[4])
                m2 = dscr(); nc.vector.scalar_tensor_tensor(m2[:, :], S[6], 2.0, m1[:, :],
                                                            op0=ALU.mult, op1=ALU.add)
                m3 = dscr(); nc.vector.tensor_mul(m3[:, :], g0t[:, :], m2[:, :])
                s2 = dt("s2"); nc.vector.tensor_tensor(out=s2[:, :], in0=m3[:, :], in1=S[5], op=ALU.add)
                m2v = dscr(); nc.vector.scalar_tensor_tensor(m2v[:, :], s0[:, :], 1.0 / 4096.0, s0[:, :],
                                                             op0=ALU.mult, op1=ALU.mult)
                var = dscr(); nc.vector.scalar_tensor_tensor(var[:, :], s2[:, :], 1.0 / 64.0, m2v[:, :],
                                                             op0=ALU.mult, op1=ALU.subtract)
                lnv = dscr(); nc.scalar.activation(lnv[:, :], var[:, :], AF.Ln, bias=c_epsln[:, :])
                rstd = dt("rstd"); nc.scalar.activation(rstd[:, :], lnv[:, :], AF.Exp, scale=-0.5)
                num1 = dscr(); nc.vector.scalar_tensor_tensor(num1[:, :], s0[:, :], -gps64, s1[:, :],
                                                              op0=ALU.mult, op1=ALU.add)
                abT = dt("abT"); nc.vector.tensor_mul(abT[:, :], num1[:, :], rstd[:, :])

                # softmax over j (partition blocks) then exp, all in [j, i] layout
                exp1 = ph2.tile([128, T], BF16, tag="exp1")
                nc.scalar.activation(exp1[:, :], abT[:, :], AF.Exp)
                den_ps = psD.tile([32, RPC], F32, tag="scrD", name="den_ps")
                for jb in range(4):
                    nc.tensor.matmul(den_ps[:, :], onesM[:, :],
                                     exp1[:, 128 * jb:128 * jb + 128],
                                     start=(jb == 0), stop=(jb == 3))
                rdenp = ph2.tile([128, RPC], F32, tag="rdenp")
                nc.vector.memset(rdenp[:, :], 0.0)
                nc.vector.reciprocal(rdenp[0:1, :], den_ps[0:1, :])
                bc_ps = psD.tile([128, RPC], F32, tag="scrD", name="bc_ps")
                nc.tensor.matmul(bc_ps[:, :], onesB[:, :], rdenp[:, :], start=True, stop=True)
                asc = ph2.tile([128, T], F32, tag="asc")
                for jb in range(4):
                    nc.vector.tensor_mul(asc[:, 128 * jb:128 * jb + 128],
                                         exp1[:, 128 * jb:128 * jb + 128], bc_ps[:, :])
                AT_sb = ph2.tile([128, T], BF16, tag="AT_sb")
                nc.scalar.activation(AT_sb[:, :], asc[:, :], AF.Exp)

                # ---- Phase D tail: attention + FFN ----
                num_ps = psD2.tile([H, RPC], F32, tag="num_ps")
                adn_ps = psD2.tile([H, RPC], F32, tag="adn_ps")
                for jb in range(4):
                    nc.tensor.matmul(num_ps[:, :], ekv[:, 128 * jb:128 * jb + 128],
                                     AT_sb[:, 128 * jb:128 * jb + 128],
                                     start=(jb == 0), stop=(jb == 3))
                for jb in range(4):
                    nc.tensor.matmul(adn_ps[:, :], eKT[:, 128 * jb:128 * jb + 128],
                                     AT_sb[:, 128 * jb:128 * jb + 128],
                                     start=(jb == 0), stop=(jb == 3))
                rden2 = phD.tile([H, RPC], F32, tag="rden2")
                nc.vector.reciprocal(rden2[:, :], adn_ps[:, :])
                attT = phD.tile([H, RPC], BF16, tag="attT")
                nc.vector.tensor_mul(attT[:, :], num_ps[:, :], rden2[:, :])
                nc.vector.tensor_mul(attT[:, :], attT[:, :], qs_sb[:, :])

                att_ps = psD2.tile([RPC, D], F32, tag="att_ps")
                nc.tensor.matmul(att_ps[:, :], attT[:, :], wo[:, :], start=True, stop=True)
                r1 = phD.tile([RPC, D], F32, tag="r1")
                nc.vector.tensor_add(r1[:, :], att_ps[:, :], xrows[:, :])

                sq1 = phD.tile([RPC, D], F32, tag="sq1")
                nc.gpsimd.tensor_mul(sq1[:, :], r1[:, :], r1[:, :])
                ms = phD.tile([RPC, 1], F32, tag="ms")
                nc.vector.reduce_sum(ms[:, :], sq1[:, :], axis=AX.X)
                nc.scalar.activation(ms[:, :], ms[:, :], AF.Ln, scale=1.0 / D, bias=c_epsrms[0:RPC, :])
                nc.scalar.activation(ms[:, :], ms[:, :], AF.Exp, scale=-0.5)
                h1 = phD.tile([RPC, D], F32, tag="h1")
                nc.vector.tensor_scalar_mul(h1[:, :], r1[:, :], ms[:, :])
                nc.vector.tensor_mul(h1[:, :], h1[:, :], rms1b[:, :])

                h1T_ps = psD.tile([D, RPC], F32, tag="scrD")
                nc.tensor.transpose(h1T_ps[:, :], h1[:, :], ident[:, :])
                h1T = phD.tile([D, RPC], BF16, tag="h1T")
                nc.vector.tensor_copy(out=h1T[:, :], in_=h1T_ps[:, :])

                relu_sb = phD.tile([128, FF], BF16, tag="relu_sb")
                for fb in range(4):
                    f_ps = psD.tile([128, RPC], F32, tag="scrD")
                    nc.tensor.matmul(f_ps[:, :], ffw1[:, 128 * fb:128 * fb + 128], h1T[:, :],
                                     start=True, stop=True)
                    nc.scalar.activation(relu_sb[:, 128 * fb:128 * fb + 128], f_ps[:, :],
                                         AF.Relu, bias=ffb1s[:, fb:fb + 1])
                o2_ps = psD2.tile([RPC, D], F32, tag="o2_ps")
                for fb in range(4):
                    nc.tensor.matmul(o2_ps[:, :], relu_sb[:, 128 * fb:128 * fb + 128],
                                     ffw2[:, 128 * fb:128 * fb + 128],
                                     start=(fb == 0), stop=(fb == 3))
                r2 = phD.tile([RPC, D], F32, tag="r2")
                nc.vector.tensor_add(r2[:, :], o2_ps[:, :], h1[:, :])
                nc.vector.tensor_add(r2[:, :], r2[:, :], ffb2b[:, :])

                sq2 = phD.tile([RPC, D], F32, tag="sq2")
                nc.gpsimd.tensor_mul(sq2[:, :], r2[:, :], r2[:, :])
                ms2 = phD.tile([RPC, 1], F32, tag="ms2")
                nc.vector.reduce_sum(ms2[:, :], sq2[:, :], axis=AX.X)
                nc.scalar.activation(ms2[:, :], ms2[:, :], AF.Ln, scale=1.0 / D, bias=c_epsrms[0:RPC, :])
                nc.scalar.activation(ms2[:, :], ms2[:, :], AF.Exp, scale=-0.5)
                outp = phD.tile([RPC, D], F32, tag="outp")
                nc.vector.tensor_scalar_mul(outp[:, :], r2[:, :], ms2[:, :])
                nc.vector.tensor_mul(outp[:, :], outp[:, :], rms2b[:, :])
                nc.sync.dma_start(out=tt["t_out"].ap()[:, :], in_=outp[:, :])

    nc.finalize()
    return nc


def _prepare(inputs):
    """Host-side: fold weights, build per-core input maps."""
    f = {k: np.asarray(v, dtype=np.float32) for k, v in inputs.items()}
    s0, s1 = float(np.exp(f["log_scale"][0])), float(np.exp(f["log_scale"][1]))
    w1 = f["mlp_w1"]
    w1c0 = (w1 * s0).copy()
    w1c1 = (w1 * s1).copy()
    w1c0[0, :] /= EPSX
    w1c1[0, :] /= EPSX
    w1big = np.zeros((128, 128), np.float32)
    for c in range(4):
        w1big[32 * c:32 * c + 9, 0:64] = w1c0
        w1big[32 * c + 9:32 * c + 18, 64:128] = w1c1

    sc18 = np.array([EPSX, 1, 1, 2, 2, 4, 4, 8, 8] * 2, np.float32)
    sb18 = np.array([0] + [0, PI / 2] * 4, np.float32)
    sb18 = np.concatenate([sb18, sb18])
    sinscale = np.ones((128, 1), np.float32)
    sinbias = np.zeros((128, 1), np.float32)
    for c in range(4):
        sinscale[32 * c:32 * c + 18, 0] = sc18
        sinbias[32 * c:32 * c + 18, 0] = sb18

    b1s = np.concatenate([f["mlp_b1"], f["mlp_b1"]]).reshape(128, 1)

    g0, g1 = f["film_gamma"][0], f["film_gamma"][1]
    be0, be1 = f["film_beta"][0], f["film_beta"][1]
    b2 = f["mlp_b2"]
    b2e = np.concatenate([g0 * b2 + be0, g1 * b2 + be1])
    if np.any(b2e != 0):
        raise NotImplementedError("nonzero folded e-bias not supported")
    W2blk = np.zeros((128, 128), np.float32)
    W2blk[0:64, 0:64] = f["mlp_w2"] * g0[None, :]
    W2blk[64:128, 64:128] = f["mlp_w2"] * g1[None, :]

    glW = (W2blk @ f["gate_w1"]).astype(np.float32)          # (128, 64)
    glWE = np.zeros((128, 128), np.float32); glWE[:, 0:64] = glW
    glWO = np.zeros((128, 128), np.float32); glWO[:, 64:128] = glW
    gb1 = f["gate_b1"].reshape(64, 1)
    gb1s = np.concatenate([gb1, gb1]).reshape(128, 1)
    temp = float(np.exp(f["gate_temp"]))
    gw2d = (f["gate_w2"][:, 0] - f["gate_w2"][:, 1]) / temp
    gb2d = float((f["gate_b2"][0] - f["gate_b2"][1]) / temp)
    gp = f["ln_g"] * f["proj_w"][:, 0]
    gps64 = float(gp.sum() / 64.0)
    if float(f["proj_b"][0]) != 0.0 or np.any(f["ln_b"] != 0):
        # ln_b contributes ln_b @ proj_w (a constant) and proj_b adds directly;
        # both shift adapt_bias by a constant -> softmax-invariant. Safe to drop.
        pass

    ones64 = np.ones(64, np.float32)
    vA0 = W2blk[:, 0:64] @ ones64
    vA1 = W2blk[:, 64:128] @ ones64
    vB0 = W2blk[:, 0:64] @ gp
    vB1 = W2blk[:, 64:128] @ gp
    # 6 blocks of [128,32], stats in cols 0..7; chunk c's band is psum
    # partitions 32c..32c+32 (4 concurrent 128x32 col-tiles on the PE).
    statsW = np.zeros((128, 192), np.float32)
    statsW[:, 0] = vA0 - vA1                           # S0 = a0 - a1
    statsW[:, 1] = vA1                                 # S1 = a1
    statsW[:, 2] = vB0 - vB1                           # S2 = b0 - b1
    statsW[:, 3] = vB1                                 # S3 = b1
    statsW[:, 32 + 4] = 1.0                            # q00 + q11 (toward S4)
    statsW[64:128, 32 + 5] = 1.0                       # S5 = q11
    statsW[64:128, 32 + 6] = -1.0                      # -q11 (toward S6)
    statsW[0:64, 64 + 4] = -2.0                        # -2qc (crx even half)
    statsW[0:64, 64 + 6] = 1.0                         # +qc
    statsW[64:128, 96 + 4] = -2.0                      # -2qc (crx odd half)
    statsW[64:128, 96 + 6] = 1.0                       # +qc
    statsW[0:64, 128 + 7] = gw2d                       # d (gg even half)
    statsW[64:128, 160 + 7] = gw2d                     # d (gg odd half)

    selT = np.zeros((128, 32), np.float32)
    for c in range(4):
        for s in range(8):
            selT[32 * c + s, 8 * c + s] = 1.0

    selC = np.zeros((128, 4096), np.float32)
    selA = np.zeros((128, 4096), np.float32)
    for q in range(NQ):
        for c in range(4):
            i = 4 * q + c
            for r in range(9):
                selC[i, 128 * q + 32 * c + r] = 1.0
                selA[i, 128 * q + 32 * c + 9 + r] = 1.0

    packF = np.zeros((128, 1033), np.float32)
    packF[:, 0] = sinscale[:, 0]
    packF[:, 1] = sinbias[:, 0]
    packF[:, 2] = b1s[:, 0]
    packF[:, 3] = gb1s[:, 0]
    packF[:, 4] = f["bq"]
    packF[:, 5:9] = f["ff_b1"].reshape(4, 128).T
    packF[:, 9:137] = np.eye(128, dtype=np.float32)
    packF[:, 137:265] = 1.0                              # onesB
    packF[:, 265:393] = np.broadcast_to(f["bv"], (128, H))
    # 393:521 = xrows (per-core, filled below)
    packF[:, 521:649] = np.broadcast_to(f["bo"], (RPC, D))
    packF[:, 649:777] = np.broadcast_to(f["rms1"], (RPC, D))
    packF[:, 777:905] = np.broadcast_to(f["rms2"], (RPC, D))
    packF[:, 905:1033] = np.broadcast_to(f["ff_b2"], (RPC, D))

    packB = np.zeros((128, 3456), np.float32)
    # 0:512 xT, 512:640 xrowsT, 640:1152 cost (per-core, filled below)
    packB[:, 1152:1280] = w1big
    packB[:, 1280:1408] = W2blk
    packB[:, 1408:1536] = glWE
    packB[:, 1536:1664] = glWO
    packB[:, 1664:1856] = statsW
    packB[:, 1856:1888] = selT
    packB[:, 1888:1920] = 1.0                            # onesM
    packB[:, 1920:2048] = f["wq"]
    packB[:, 2048:2176] = f["wk"]
    packB[:, 2176:2304] = f["wv"]
    packB[:, 2304:2432] = f["wo"]
    packB[:, 2432:2944] = f["ff_w1"]
    packB[:, 2944:3456] = np.concatenate(
        [f["ff_w2"][128 * fb:128 * fb + 128, :] for fb in range(4)], axis=1)

    shared = {
        "selC": selC.astype(BF), "selA": selA.astype(BF),
    }

    in_maps = []
    for core in range(NCORES):
        b = core // 4
        r0 = (core % 4) * RPC
        xb = f["x"][b]
        cx = f["coords"][b, :, 0]
        cy = f["coords"][b, :, 1]
        pF = packF.copy()
        pF[:, 393:521] = xb[r0:r0 + RPC] + f["bo"][None, :]
        pB = packB.copy()
        pB[:, 0:512] = xb.T
        pB[:, 512:640] = xb[r0:r0 + RPC].T
        pB[:, 640:1152] = f["cost_mat"][b, r0:r0 + RPC]
        p2 = np.zeros((2, 1280), np.float32)
        p2[:, 0:RPC] = np.stack([cx[r0:r0 + RPC], np.ones(RPC, np.float32)])
        p2[:, RPC:2 * RPC] = np.stack([cy[r0:r0 + RPC], np.ones(RPC, np.float32)])
        p2[:, 256:256 + T] = np.stack([np.ones(T, np.float32), -cx])
        p2[:, 768:768 + T] = np.stack([np.ones(T, np.float32), -cy])
        m = dict(shared)
        m.update({
            "pack2": p2,
            "packF": pF,
            "packB": pB.astype(BF),
        })
        in_maps.append(m)
    return in_maps, {"gb2d": gb2d, "gps64": gps64}


def _get_program(consts):
    key = tuple(sorted(consts.items()))
    if key not in _CACHE:
        nc, loc = _build_program()
        tt = {k: v for k, v in loc.items() if k.startswith("t_")}
        nc = _emit(nc, tt, consts)
        _CACHE[key] = nc
    return _CACHE[key]


def kernel(**inputs):
    in_maps, consts = _prepare(inputs)
    nc = _get_program(consts)
    res = bass_utils.run_bass_kernel_spmd(nc, in_maps, core_ids=list(range(NCORES)))
    out = np.zeros((B, T, D), np.float32)
    for core in range(NCORES):
        b = core // 4
        r0 = (core % 4) * RPC
        out[b, r0:r0 + RPC] = res.results[core]["out"]
    return out



# revision 60
# speedup vs baseline: 1.0931x; 1.0931x over previous
"""Trainium2 Bass kernel for nn_AttnFree_Block (AFT + neural adaptive pairwise bias).

Sharding: 8 cores over the (B=2, T=512) query-row grid -> 128 query rows/core.
Each core computes the full pairwise bias network for its 128 rows x 512 keys,
then the AFT attention + FFN for its rows. Output rows are gathered on host.

v3 restructure vs v2 (332us -> 270us):
- stats matmuls col-tiled 4-way: each chunk's 8 stats accumulate in its own
  32-partition PSUM band via tile_position=(0,32c); the four 128x32 PE tiles
  run concurrently, cutting stats matmul time ~4x.
- gate-hidden matmuls col-tiled 2-way (two concurrent 128x64 tiles/pair).
- h matmuls row-tiled (32-strip weights) run concurrently per pair.
- e matmul + evac done pair-wide ([128,1024] tiles, one wide cast); the e/h
  PSUM tiles share one 2-buf rotating pool (h0,e0,h1,e1 ping-pong).
- qc = e0*e1 in one wide [64,1024] mul into static base-0 tiles (rows 64:128
  zeroed once) so a single stats weight block serves both chunk parities.
- scalar FIFO spine: sin(q+1) emitted right after silu(q,0) so the
  sin->h->silu cycle never waits behind gl-silu/evac work.
- gate sigmoid linearized (|x|<=0.06 -> 0.5+x/4, 5e-6 abs) -> no table swap.
- stats transpose-evac via bf16 selector matmul; inputs packed into 3 wide
  DMAs instead of ~40 small ones.
- softmax(adapt_bias) computed in transposed [j, i] layout -> no A transposes.
"""
import sys
sys.path.insert(0, '/opt/trn_rl_repo')

import math
import numpy as np
import ml_dtypes

BF = ml_dtypes.bfloat16

import concourse.bass as bass
import concourse.bacc as bacc
import concourse.mybir as mybir
import concourse.tile as tile
from concourse import bass_utils

F32 = mybir.dt.float32
F32R = mybir.dt.float32r
BF16 = mybir.dt.bfloat16

AF = mybir.ActivationFunctionType
ALU = mybir.AluOpType
AX = mybir.AxisListType

B, T, D, H, HID, FF = 2, 512, 128, 128, 64, 512
NCORES = 8
RPC = T * B // NCORES  # 128 query rows per core
NQ = RPC // 4          # 32 quads
EPS_LN = 1e-5
EPS_RMS = 1e-5
EPSX = 1e-4            # epsilon for the sin(eps*x)/eps ~= x identity-row trick
PI = math.pi

_CACHE = {}


def _build_program():
    nc = bacc.Bacc()

    def din(name, shape, dt=F32):
        return nc.dram_tensor(name, list(shape), dt, kind="ExternalInput")

    # consolidated input packs (few big DMAs instead of ~40 small ones)
    t_pack2 = din("pack2", (2, 1280))          # clx|cly|crx|cry
    t_packF = din("packF", (128, 1033))        # f32 constants, see PF offsets
    t_packB = din("packB", (128, 3456), BF16)  # bf16 weights, see PB offsets
    t_selC = din("selC", (128, 4096), BF16)   # per-quad one-hot row selectors (cost)
    t_selA = din("selA", (128, 4096), BF16)   # per-quad one-hot row selectors (angle)

    t_out = nc.dram_tensor("out", [RPC, D], F32, kind="ExternalOutput")
    return nc, locals()


def _emit(nc, tt, consts):
    gb2d = consts["gb2d"]
    gps64 = consts["gps64"]

    with tile.TileContext(nc) as tc:
        import contextlib
        with contextlib.ExitStack() as ctx:
            singles = ctx.enter_context(tc.tile_pool(name="singles", bufs=1))
            sb = ctx.enter_context(tc.tile_pool(name="sb", bufs=1))

            _eng_rr = [nc.gpsimd, nc.scalar, nc.sync]

            def load1(t, shape, pool=None, eng=None):
                p = pool or singles
                s = p.tile(list(shape), t.dtype, tag=t.name + "_sb", name=t.name + "_sb")
                e = eng if eng is not None else _eng_rr[load1._i % 3]
                load1._i += 1
                e.dma_start(out=s[:, :], in_=t.ap()[:, :])
                return s
            load1._i = 0

            # ---- consolidated loads: a handful of wide DMAs ----
            pack2 = singles.tile([2, 1280], F32, tag="pack2_sb", name="pack2_sb")
            nc.sync.dma_start(out=pack2[:, :], in_=tt["t_pack2"].ap()[:, :])
            packB = singles.tile([128, 3456], BF16, tag="packB_sb", name="packB_sb")
            nc.scalar.dma_start(out=packB[:, 0:1728],
                                in_=tt["t_packB"].ap()[:, 0:1728])
            nc.scalar.dma_start(out=packB[:, 1728:3456],
                                in_=tt["t_packB"].ap()[:, 1728:3456])
            packF = singles.tile([128, 1033], F32, tag="packF_sb", name="packF_sb")
            nc.gpsimd.dma_start(out=packF[:, 0:517], in_=tt["t_packF"].ap()[:, 0:517])
            nc.gpsimd.dma_start(out=packF[:, 517:1033],
                                in_=tt["t_packF"].ap()[:, 517:1033])
            selC = singles.tile([128, 4096], BF16, tag="selC_sb", name="selC_sb")
            selA = singles.tile([128, 4096], BF16, tag="selA_sb", name="selA_sb")
            for hh in range(2):
                nc.sync.dma_start(out=selC[:, 2048 * hh:2048 * hh + 2048],
                                  in_=tt["t_selC"].ap()[:, 2048 * hh:2048 * hh + 2048])
                nc.gpsimd.dma_start(out=selA[:, 2048 * hh:2048 * hh + 2048],
                                    in_=tt["t_selA"].ap()[:, 2048 * hh:2048 * hh + 2048])

            clx = pack2[:, 0:RPC]
            cly = pack2[:, RPC:2 * RPC]
            crx = pack2[:, 256:256 + T]
            cry = pack2[:, 768:768 + T]
            sinscale = packF[:, 0:1]
            sinbias = packF[:, 1:2]
            b1s = packF[:, 2:3]
            gb1s = packF[:, 3:4]
            bq = packF[:, 4:5]
            ffb1s = packF[:, 5:9]
            ident = packF[:, 9:137]
            onesB = packF[:, 137:265]
            bvb = packF[:, 265:393]
            xrows = packF[:, 393:521]
            bob = packF[:, 521:649]
            rms1b = packF[:, 649:777]
            rms2b = packF[:, 777:905]
            ffb2b = packF[:, 905:1033]
            xT = packB[:, 0:512]
            xrowsT = packB[:, 512:640]
            cost_sb = packB[:, 640:1152]
            w1big = packB[:, 1152:1280]
            w2x = packB[:, 1280:1408]
            glWE = packB[:, 1408:1536]
            glWO = packB[:, 1536:1664]
            statsW = packB[:, 1664:1856]
            selT = packB[:, 1856:1888]
            onesM = packB[:, 1888:1920]
            wq = packB[:, 1920:2048]
            wk = packB[:, 2048:2176]
            wv = packB[:, 2176:2304]
            wo = packB[:, 2304:2432]
            ffw1 = packB[:, 2432:2944]
            ffw2 = packB[:, 2944:3456]

            # static double-buffered crx tiles: qc products live in partitions
            # 0:64 (cols 0:T = even chunk, T:2T = odd); rows 64:128 are zeroed
            # once so the stats matmul can read all 128 partitions safely.
            mixT = []
            for mb in range(4):
                mt = singles.tile([128, 2 * T], BF16, tag=f"mixT{mb}")
                nc.gpsimd.memset(mt[64:128, :], 0.0)
                mixT.append(mt)

            c_epsln = singles.tile([128, 1], F32, tag="c_epsln")
            nc.vector.memset(c_epsln[:, :], EPS_LN)
            c_epsrms = singles.tile([128, 1], F32, tag="c_epsrms")
            nc.vector.memset(c_epsrms[:, :], EPS_RMS)
            c_gb2d = singles.tile([128, 1], F32, tag="c_gb2d")
            nc.vector.memset(c_gb2d[:, :], gb2d)

            # ================= Phase A: angle matrix =================
            with tc.tile_pool(name="pA", bufs=2, space="PSUM") as pA, \
                 tc.tile_pool(name="paS", bufs=1) as paS:
                dx_ps = pA.tile([RPC, T], F32, tag="dxy")
                nc.tensor.matmul(dx_ps[:, :], clx[:, :], crx[:, :], start=True, stop=True)
                dy_ps = pA.tile([RPC, T], F32, tag="dxy")
                nc.tensor.matmul(dy_ps[:, :], cly[:, :], cry[:, :], start=True, stop=True)
                negx = paS.tile([RPC, T], F32, tag="negx")
                nc.vector.tensor_scalar(negx[:, :], dx_ps[:, :], -1.0, None, op0=ALU.mult)
                dxa = paS.tile([RPC, T], F32, tag="dxa")
                nc.vector.tensor_tensor(out=dxa[:, :], in0=negx[:, :], in1=dx_ps[:, :], op=ALU.max)
                cmpneg = paS.tile([RPC, T], F32, tag="cmpneg")
                nc.vector.tensor_scalar(cmpneg[:, :], dx_ps[:, :], 0.0, None, op0=ALU.is_lt)
                sdy = paS.tile([RPC, T], F32, tag="sdy")
                nc.vector.tensor_scalar(sdy[:, :], dy_ps[:, :], 0.0, None, op0=ALU.is_ge)
                nc.vector.tensor_scalar(sdy[:, :], sdy[:, :], 2.0, -1.0, op0=ALU.mult, op1=ALU.add)
                dya = paS.tile([RPC, T], F32, tag="dya")
                nc.vector.tensor_mul(dya[:, :], sdy[:, :], dy_ps[:, :])
                mx = paS.tile([RPC, T], F32, tag="mx")
                nc.vector.tensor_tensor(out=mx[:, :], in0=dxa[:, :], in1=dya[:, :], op=ALU.max)
                nc.vector.tensor_scalar_max(mx[:, :], mx[:, :], 1e-30)
                mn = paS.tile([RPC, T], F32, tag="mn")
                nc.vector.tensor_tensor(out=mn[:, :], in0=dxa[:, :], in1=dya[:, :], op=ALU.min)
                nc.vector.reciprocal(mx[:, :], mx[:, :])
                rt = paS.tile([RPC, T], F32, tag="rt")
                nc.vector.tensor_mul(rt[:, :], mn[:, :], mx[:, :])
                at = paS.tile([RPC, T], F32, tag="at")
                nc.scalar.activation(at[:, :], rt[:, :], AF.Arctan)
                swap = paS.tile([RPC, T], F32, tag="swap")
                nc.vector.tensor_tensor(out=swap[:, :], in0=dya[:, :], in1=dxa[:, :], op=ALU.is_gt)
                v1 = paS.tile([RPC, T], F32, tag="v1")
                nc.vector.tensor_scalar(v1[:, :], at[:, :], -2.0, PI / 2, op0=ALU.mult, op1=ALU.add)
                nc.gpsimd.tensor_mul(v1[:, :], v1[:, :], swap[:, :])
                base = paS.tile([RPC, T], F32, tag="base")
                nc.gpsimd.tensor_add(base[:, :], at[:, :], v1[:, :])
                v2 = paS.tile([RPC, T], F32, tag="v2")
                nc.vector.tensor_scalar(v2[:, :], base[:, :], -2.0, PI, op0=ALU.mult, op1=ALU.add)
                nc.gpsimd.tensor_mul(v2[:, :], v2[:, :], cmpneg[:, :])
                nc.gpsimd.tensor_add(base[:, :], base[:, :], v2[:, :])
                angle = sb.tile([RPC, T], BF16, tag="angle")
                nc.vector.tensor_mul(angle[:, :], sdy[:, :], base[:, :])

                # ---- Phase D head (independent of bias net): runs in the
                # phase-A window while the PE is otherwise idle ----
                with tc.tile_pool(name="psDH", bufs=2, space="PSUM") as psDH:
                    kl_ps = pA.tile([H, T], F32, tag="dxy", name="kl_ps")
                    nc.tensor.matmul(kl_ps[:, :], wk[:, :], xT[:, :], start=True, stop=True)
                    q_ps = psDH.tile([H, RPC], F32, tag="scrDH", name="q_ps")
                    nc.tensor.matmul(q_ps[:, :], wq[:, :], xrowsT[:, :], start=True, stop=True)
                    qs_sb = sb.tile([H, RPC], F32, tag="qs_sb")
                    nc.scalar.activation(qs_sb[:, :], q_ps[:, :], AF.Sigmoid, bias=bq[:, :])

                    kex = paS.tile([H, T], F32, tag="kex")
                    nc.scalar.activation(kex[:, :], kl_ps[:, :], AF.Exp)
                    ksum = paS.tile([H, 1], F32, tag="ksum")
                    nc.vector.reduce_sum(ksum[:, :], kex[:, :], axis=AX.X)
                    nc.vector.reciprocal(ksum[:, :], ksum[:, :])
                    K_sb = paS.tile([H, T], F32, tag="K_sb")
                    nc.vector.tensor_scalar_mul(K_sb[:, :], kex[:, :], ksum[:, :])
                    eK = paS.tile([H, T], F32, tag="eK")
                    nc.scalar.activation(eK[:, :], K_sb[:, :], AF.Exp)
                    eKT = sb.tile([128, T], BF16, tag="eKT")
                    for tb in range(4):
                        tp = psDH.tile([128, 128], F32, tag="scrDH", name="tp")
                        nc.tensor.transpose(tp[:, :], eK[:, 128 * tb:128 * tb + 128], ident[:, :])
                        nc.vector.tensor_copy(out=eKT[:, 128 * tb:128 * tb + 128], in_=tp[:, :])
                    ekv = sb.tile([128, T], BF16, tag="ekv")
                    for tb in range(4):
                        v_ps = psDH.tile([128, H], F32, tag="scrDH", name="v_ps")
                        nc.tensor.matmul(v_ps[:, :], xT[:, 128 * tb:128 * tb + 128], wv[:, :],
                                         start=True, stop=True)
                        vb = paS.tile([128, H], BF16, tag="vb", name="vb")
                        nc.vector.tensor_add(vb[:, :], v_ps[:, :], bvb[:, :])
                        nc.vector.tensor_mul(ekv[:, 128 * tb:128 * tb + 128], vb[:, :],
                                             eKT[:, 128 * tb:128 * tb + 128])

            # ================= Phase B: bias-net main loop =================
            T_coll = singles.tile([128, 4096], F32, tag="Tcoll")
            # s-major layout: col = s*512 + jb*128 + i  -> S[k] slices are contiguous
            Tout = T_coll[:, :].rearrange("p (s jb i) -> p jb i s", s=8, jb=4, i=128)
            S = [T_coll[:, 512 * k:512 * k + 512] for k in range(8)]

            with tc.tile_pool(name="feS", bufs=2) as feS, \
                 tc.tile_pool(name="hS", bufs=6) as hS, \
                 tc.tile_pool(name="eS", bufs=4) as eS, \
                 tc.tile_pool(name="hzS", bufs=8) as hzS, \
                 tc.tile_pool(name="ggS", bufs=4) as ggS, \
                 tc.tile_pool(name="evS", bufs=2) as evS, \
                 tc.tile_pool(name="psFS", bufs=2, space="PSUM") as psFS, \
                 tc.tile_pool(name="psHE", bufs=2, space="PSUM") as psHE, \
                 tc.tile_pool(name="psGL", bufs=1, space="PSUM") as psGL, \
                 tc.tile_pool(name="psTR", bufs=1, space="PSUM") as psTR:

                state = {}

                def produce_quad(q):
                    # fe for the 4 chunks of quad q
                    fe_ps = psFS.tile([128, T], F32, tag="fs_ps", name="fe_ps")
                    nc.tensor.matmul(fe_ps[:, :], selC[:, 128 * q:128 * q + 128],
                                     cost_sb[:, :], start=True, stop=False)
                    nc.tensor.matmul(fe_ps[:, :], selA[:, 128 * q:128 * q + 128],
                                     angle[:, :], start=False, stop=True)
                    fe_sb = feS.tile([128, T], BF16, tag="fe_sb", name="fe_sb")
                    nc.scalar.activation(fe_sb[:, :], fe_ps[:, :], AF.Sin,
                                         scale=sinscale[:, :], bias=sinbias[:, :])
                    state[("fe", q)] = fe_sb

                def h_mms(q, k):
                    fe_sb = state[("fe", q)]
                    h2_ps = psHE.tile([128, 2 * T], F32, tag="he_ps", name="h2_ps")
                    for half in (0, 1):
                        c = 2 * k + half
                        nc.tensor.matmul(h2_ps[:, T * half:T * half + T],
                                         w1big[32 * c:32 * c + 18, :],
                                         fe_sb[32 * c:32 * c + 18, :], start=True, stop=True,
                                         tile_position=(32 * c, 0))
                    state[("h2ps", q, k)] = h2_ps

                def pair_silu(q, k):
                    h2_ps = state.pop(("h2ps", q, k))
                    h2_sb = hS.tile([128, 2 * T], BF16, tag="h2_sb", name="h2_sb")
                    nc.scalar.activation(h2_sb[:, :], h2_ps[:, :], AF.Silu, bias=b1s[:, :])
                    state[("h2", q, k)] = h2_sb

                def e_pair(q, k):
                    # e = [W0.T h0; W1.T h1] for chunks 2k, 2k+1 into one
                    # [128, 2T] tile; one wide cast halves the V evac cost.
                    h2 = state[("h2", q, k)]
                    e2_ps = psHE.tile([128, 2 * T], F32, tag="he_ps", name="e2_ps")
                    for half in (0, 1):
                        nc.tensor.matmul(e2_ps[:, T * half:T * half + T], w2x[:, :],
                                         h2[:, T * half:T * half + T],
                                         start=True, stop=True)
                    e2_sb = eS.tile([128, 2 * T], BF16, tag="e_sb", name="e2_sb")
                    nc.vector.tensor_copy(out=e2_sb[:, :], in_=e2_ps[:, :])
                    state[("e2ps", q, k)] = e2_ps
                    state[("e2", q, k)] = e2_sb

                def sq_pair(q, k):
                    e2_sb = state[("e2", q, k)]
                    sq_sb = hzS.tile([128, 2 * T], BF16, tag="hz_sb", name="sq_sb")
                    nc.gpsimd.tensor_mul(sq_sb[:, :], e2_sb[:, :], e2_sb[:, :])
                    state[("hz2", q, k)] = sq_sb

                def crx_mul(q, k):
                    # qc products for both chunks of pair k in one wide op:
                    # mixT[0:64, 0:2T] = e0 * e1 (psum x sbuf, partitions 0:64)
                    crx_sb = mixT[(2 * q + k) % 4]
                    e2_ps = state.pop(("e2ps", q, k))
                    e2_sb = state.pop(("e2", q, k))
                    nc.vector.tensor_mul(crx_sb[0:64, :],
                                         e2_ps[0:64, :], e2_sb[64:128, :])
                    state[("mix", q, k)] = crx_sb

                def pair_gl_mms(q, k):
                    # gate-hidden for both chunks of pair k: two concurrent
                    # 128x64 col-tiles (even chunk -> rows 0:64, odd -> 64:128)
                    gl_ps = psGL.tile([128, T], F32, tag="gl_ps", name="gl_ps")
                    h2 = state[("h2", q, k)]
                    nc.tensor.matmul(gl_ps[0:64, :], glWE[:, 0:64], h2[:, 0:T],
                                     start=True, stop=True, tile_position=(0, 0),
                                     skip_group_check=True)
                    nc.tensor.matmul(gl_ps[64:128, :], glWO[:, 64:128], h2[:, T:2 * T],
                                     start=True, stop=True, tile_position=(0, 64),
                                     skip_group_check=True)
                    state[("glps", q, k)] = gl_ps

                def gl_silu(q, k):
                    gl_ps = state.pop(("glps", q, k))
                    gg_sb = ggS.tile([128, T], BF16, tag="gg_sb", name="gg_sb")
                    nc.scalar.activation(gg_sb[:, :], gl_ps[:, :], AF.Silu, bias=gb1s[:, :])
                    state[("gg", q, k)] = gg_sb

                # ---- stats: 4 concurrent 128x32 col-tiles, one per chunk ----
                # band c = psum partitions 32c..32c+32; stats at rows 32c+s.
                # statsW blocks: 0=h, 1=sq, 2=crx-even, 3=crx-odd, 4=gg-even,
                # 5=gg-odd (each [128,32] with stats in cols 0..7).
                def stats_round_h(q):
                    st_ps = psFS.tile([128, T], F32, tag="fs_ps", name="st_ps")
                    for c in range(4):
                        h_sl = state[("h2", q, c // 2)][:, T * (c % 2):T * (c % 2) + T]
                        nc.tensor.matmul(st_ps[32 * c:32 * c + 32, :],
                                         statsW[:, 0:32], h_sl,
                                         start=True, stop=False,
                                         tile_position=(0, 32 * c),
                                         skip_group_check=True)
                    state[("st", q)] = st_ps

                def stats_round_sq(q):
                    st_ps = state[("st", q)]
                    for c in range(4):
                        sq2 = state[("hz2", q, c // 2)]
                        nc.tensor.matmul(st_ps[32 * c:32 * c + 32, :],
                                         statsW[:, 32:64],
                                         sq2[:, T * (c % 2):T * (c % 2) + T],
                                         start=False, stop=False,
                                         tile_position=(0, 32 * c),
                                         skip_group_check=True)
                    for k in range(2):
                        state.pop(("hz2", q, k))

                def stats_round_xg(q):
                    st_ps = state[("st", q)]
                    for k in range(2):
                        mix = state.pop(("mix", q, k))
                        for par in range(2):
                            c = 2 * k + par
                            nc.tensor.matmul(st_ps[32 * c:32 * c + 32, :],
                                             statsW[:, 64:96],
                                             mix[:, T * par:T * par + T],
                                             start=False, stop=False,
                                             tile_position=(0, 32 * c),
                                             skip_group_check=True)
                    for k in range(2):
                        gg = state.pop(("gg", q, k))
                        for par in range(2):
                            c = 2 * k + par
                            nc.tensor.matmul(st_ps[32 * c:32 * c + 32, :],
                                             statsW[:, 128 + 32 * par:160 + 32 * par],
                                             gg[:, :], start=False, stop=True,
                                             tile_position=(0, 32 * c),
                                             skip_group_check=True)
                    for k in range(2):
                        state.pop(("h2", q, k))

                def evac_copy(q):
                    st_ps = state[("st", q)]
                    sts_sb = evS.tile([128, T], BF16, tag="sts_sb", name="sts_sb")
                    nc.scalar.activation(sts_sb[:, :], st_ps[:, :], AF.Copy)
                    state[("sts", q)] = sts_sb

                def evac_transpose(q):
                    state.pop(("st", q))
                    sts_sb = state[("sts", q)]
                    trp_ps = psTR.tile([128, 128], BF16, tag="trp_ps", name="trp_ps")
                    for jb in range(4):
                        # out[j, 8c+s] = sts[32c+s, 128jb+j] via selector matmul
                        nc.tensor.transpose(trp_ps[:, 32 * jb:32 * jb + 32],
                                            sts_sb[:, 128 * jb:128 * jb + 128],
                                            selT[:, :])
                    state[("trp", q)] = trp_ps

                def evac_tcoll(q):
                    trp_ps = state.pop(("trp", q))
                    state.pop(("sts", q))
                    stv = trp_ps[:, :].rearrange("p (jb c s) -> p jb c s", jb=4, c=4)
                    nc.vector.tensor_copy(out=Tout[:, :, 4 * q:4 * q + 4, 0:8],
                                          in_=stv[:, :, :, 0:8])

                # Steady-state spine: both h-MM pairs issue back-to-back, the
                # two h-silus run back-to-back on scalar, and sin(q+1) follows
                # immediately — the scalar FIFO no longer interleaves gl/evac
                # work into the latency-critical sin->h->silu->sin cycle.
                # scalar FIFO per iter: silu0, sin(q+1), gl-silu(q-1,1),
                # silu1, gl-silu(q,0), evac(q-1) -- gl1's silu is deferred one
                # iteration so the S queue never bunches up between the two
                # latency-critical h-silus.
                produce_quad(0)
                for q in range(NQ):
                    h_mms(q, 0)
                    pair_silu(q, 0)
                    if q + 1 < NQ:
                        produce_quad(q + 1)
                    if q >= 1:
                        gl_silu(q - 1, 1)
                        stats_round_h(q - 1)
                    e_pair(q, 0)
                    sq_pair(q, 0)
                    h_mms(q, 1)
                    pair_silu(q, 1)
                    crx_mul(q, 0)
                    pair_gl_mms(q, 0)
                    gl_silu(q, 0)
                    if q >= 1:
                        stats_round_sq(q - 1)
                    e_pair(q, 1)
                    sq_pair(q, 1)
                    crx_mul(q, 1)
                    pair_gl_mms(q, 1)
                    if q >= 1:
                        stats_round_xg(q - 1)
                        evac_copy(q - 1)
                        evac_transpose(q - 1)
                        evac_tcoll(q - 1)
                q = NQ
                gl_silu(q - 1, 1)
                stats_round_h(q - 1)
                stats_round_sq(q - 1)
                stats_round_xg(q - 1)
                evac_copy(q - 1)
                evac_transpose(q - 1)
                evac_tcoll(q - 1)

            # ================= Phase C/D: bias assembly + AFT + FFN =================

            with tc.tile_pool(name="ph2", bufs=1) as ph2, \
                 tc.tile_pool(name="ph2s", bufs=4) as ph2s, \
                 tc.tile_pool(name="phD", bufs=1) as phD, \
                 tc.tile_pool(name="psD", bufs=3, space="PSUM") as psD, \
                 tc.tile_pool(name="psD2", bufs=1, space="PSUM") as psD2:

                # keep the PE clock ramped through the scalar/vector-only
                # phase-C window: idle gaps reset the p-state and the whole
                # attention/FFN tail then runs at the mid clock.  ~8us of
                # dependency-free filler matmuls hold the ramp.
                warm_ps = psD.tile([128, RPC], F32, tag="scrD", name="warm_ps")
                for _w in range(110):
                    nc.tensor.matmul(warm_ps[:, :], w2x[:, :], selC[:, 0:RPC],
                                     start=True, stop=True, skip_group_check=True)

                # ---- Phase C: assemble adaptive bias in [j, i] layout ----
                def dt(tag):
                    return ph2.tile([128, T], F32, tag=tag, name=tag)

                def dscr():
                    return ph2s.tile([128, T], F32, tag="cscr", name="cscr")

                # stats arrive pre-combined: S0=a0-a1, S1=a1, S2=b0-b1, S3=b1,
                # S4=q00+q11-2qc, S5=q11, S6=qc-q11, S7=d
                # gate sigmoid linearized: |x|<=0.06 -> sigmoid(x)=0.5+x/4 (5e-6 abs)
                g0t = dt("g0t")
                nc.vector.tensor_scalar(g0t[:, :], S[7], 0.25, 0.25 * gb2d + 0.5,
                                        op0=ALU.mult, op1=ALU.add)
                s0 = dt("s0"); nc.vector.tensor_mul(s0[:, :], g0t[:, :], S[0])
                nc.vector.tensor_tensor(out=s0[:, :], in0=s0[:, :], in1=S[1], op=ALU.add)
                s1 = dt("s1"); nc.vector.tensor_mul(s1[:, :], g0t[:, :], S[2])
                nc.vector.tensor_tensor(out=s1[:, :], in0=s1[:, :], in1=S[3], op=ALU.add)
                # sum(fused^2) = q11 + g*(2*(qc - q11) + g*(q00 + q11 - 2qc))
                m1 = dscr(); nc.vector.tensor_mul(m1[:, :], g0t[:, :], S[4])
                m2 = dscr(); nc.vector.scalar_tensor_tensor(m2[:, :], S[6], 2.0, m1[:, :],
                                                            op0=ALU.mult, op1=ALU.add)
                m3 = dscr(); nc.vector.tensor_mul(m3[:, :], g0t[:, :], m2[:, :])
                s2 = dt("s2"); nc.vector.tensor_tensor(out=s2[:, :], in0=m3[:, :], in1=S[5], op=ALU.add)
                m2v = dscr(); nc.vector.scalar_tensor_tensor(m2v[:, :], s0[:, :], 1.0 / 4096.0, s0[:, :],
                                                             op0=ALU.mult, op1=ALU.mult)
                var = dscr(); nc.vector.scalar_tensor_tensor(var[:, :], s2[:, :], 1.0 / 64.0, m2v[:, :],
                                                             op0=ALU.mult, op1=ALU.subtract)
                lnv = dscr(); nc.scalar.activation(lnv[:, :], var[:, :], AF.Ln, bias=c_epsln[:, :])
                rstd = dt("rstd"); nc.scalar.activation(rstd[:, :], lnv[:, :], AF.Exp, scale=-0.5)
                num1 = dscr(); nc.vector.scalar_tensor_tensor(num1[:, :], s0[:, :], -gps64, s1[:, :],
                                                              op0=ALU.mult, op1=ALU.add)
                abT = dt("abT"); nc.vector.tensor_mul(abT[:, :], num1[:, :], rstd[:, :])

                # softmax over j (partition blocks) then exp, all in [j, i] layout
                exp1 = ph2.tile([128, T], BF16, tag="exp1")
                nc.scalar.activation(exp1[:, :], abT[:, :], AF.Exp)
                den_ps = psD.tile([32, RPC], F32, tag="scrD", name="den_ps")
                for jb in range(4):
                    nc.tensor.matmul(den_ps[:, :], onesM[:, :],
                                     exp1[:, 128 * jb:128 * jb + 128],
                                     start=(jb == 0), stop=(jb == 3))
                rdenp = ph2.tile([128, RPC], F32, tag="rdenp")
                nc.vector.memset(rdenp[:, :], 0.0)
                nc.vector.reciprocal(rdenp[0:1, :], den_ps[0:1, :])
                bc_ps = psD.tile([128, RPC], F32, tag="scrD", name="bc_ps")
                nc.tensor.matmul(bc_ps[:, :], onesB[:, :], rdenp[:, :], start=True, stop=True)
                asc = ph2.tile([128, T], F32, tag="asc")
                for jb in range(4):
                    nc.vector.tensor_mul(asc[:, 128 * jb:128 * jb + 128],
                                         exp1[:, 128 * jb:128 * jb + 128], bc_ps[:, :])
                AT_sb = ph2.tile([128, T], BF16, tag="AT_sb")
                nc.scalar.activation(AT_sb[:, :], asc[:, :], AF.Exp)

                # ---- Phase D tail: attention + FFN ----
                num_ps = psD2.tile([H, RPC], F32, tag="num_ps")
                adn_ps = psD2.tile([H, RPC], F32, tag="adn_ps")
                for jb in range(4):
                    nc.tensor.matmul(num_ps[:, :], ekv[:, 128 * jb:128 * jb + 128],
                                     AT_sb[:, 128 * jb:128 * jb + 128],
                                     start=(jb == 0), stop=(jb == 3))
                for jb in range(4):
                    nc.tensor.matmul(adn_ps[:, :], eKT[:, 128 * jb:128 * jb + 128],
                                     AT_sb[:, 128 * jb:128 * jb + 128],
                                     start=(jb == 0), stop=(jb == 3))
                rden2 = phD.tile([H, RPC], F32, tag="rden2")
                nc.vector.reciprocal(rden2[:, :], adn_ps[:, :])
                attT = phD.tile([H, RPC], BF16, tag="attT")
                nc.vector.tensor_mul(attT[:, :], num_ps[:, :], rden2[:, :])
                nc.vector.tensor_mul(attT[:, :], attT[:, :], qs_sb[:, :])

                att_ps = psD2.tile([RPC, D], F32, tag="att_ps")
                nc.tensor.matmul(att_ps[:, :], attT[:, :], wo[:, :], start=True, stop=True)
                r1 = phD.tile([RPC, D], F32, tag="r1")
                nc.vector.tensor_add(r1[:, :], att_ps[:, :], xrows[:, :])

                sq1 = phD.tile([RPC, D], F32, tag="sq1")
                nc.gpsimd.tensor_mul(sq1[:, :], r1[:, :], r1[:, :])
                ms = phD.tile([RPC, 1], F32, tag="ms")
                nc.vector.reduce_sum(ms[:, :], sq1[:, :], axis=AX.X)
                nc.scalar.activation(ms[:, :], ms[:, :], AF.Ln, scale=1.0 / D, bias=c_epsrms[0:RPC, :])
                nc.scalar.activation(ms[:, :], ms[:, :], AF.Exp, scale=-0.5)
                h1 = phD.tile([RPC, D], F32, tag="h1")
                nc.vector.tensor_scalar_mul(h1[:, :], r1[:, :], ms[:, :])
                nc.vector.tensor_mul(h1[:, :], h1[:, :], rms1b[:, :])

                h1T_ps = psD.tile([D, RPC], F32, tag="scrD")
                nc.tensor.transpose(h1T_ps[:, :], h1[:, :], ident[:, :])
                h1T = phD.tile([D, RPC], BF16, tag="h1T")
                nc.vector.tensor_copy(out=h1T[:, :], in_=h1T_ps[:, :])

                relu_sb = phD.tile([128, FF], BF16, tag="relu_sb")
                for fb in range(4):
                    f_ps = psD.tile([128, RPC], F32, tag="scrD")
                    nc.tensor.matmul(f_ps[:, :], ffw1[:, 128 * fb:128 * fb + 128], h1T[:, :],
                                     start=True, stop=True)
                    nc.scalar.activation(relu_sb[:, 128 * fb:128 * fb + 128], f_ps[:, :],
                                         AF.Relu, bias=ffb1s[:, fb:fb + 1])
                o2_ps = psD2.tile([RPC, D], F32, tag="o2_ps")
                for fb in range(4):
                    nc.tensor.matmul(o2_ps[:, :], relu_sb[:, 128 * fb:128 * fb + 128],
                                     ffw2[:, 128 * fb:128 * fb + 128],
                                     start=(fb == 0), stop=(fb == 3))
                r2 = phD.tile([RPC, D], F32, tag="r2")
                nc.vector.tensor_add(r2[:, :], o2_ps[:, :], h1[:, :])
                nc.vector.tensor_add(r2[:, :], r2[:, :], ffb2b[:, :])

                sq2 = phD.tile([RPC, D], F32, tag="sq2")
                nc.gpsimd.tensor_mul(sq2[:, :], r2[:, :], r2[:, :])
                ms2 = phD.tile([RPC, 1], F32, tag="ms2")
                nc.vector.reduce_sum(ms2[:, :], sq2[:, :], axis=AX.X)
                nc.scalar.activation(ms2[:, :], ms2[:, :], AF.Ln, scale=1.0 / D, bias=c_epsrms[0:RPC, :])
                nc.scalar.activation(ms2[:, :], ms2[:, :], AF.Exp, scale=-0.5)
                outp = phD.tile([RPC, D], F32, tag="outp")
                nc.vector.tensor_scalar_mul(outp[:, :], r2[:, :], ms2[:, :])
                nc.vector.tensor_mul(outp[:, :], outp[:, :], rms2b[:, :])
                nc.sync.dma_start(out=tt["t_out"].ap()[:, :], in_=outp[:, :])

    nc.finalize()
    return nc


def _prepare(inputs):
    """Host-side: fold weights, build per-core input maps."""
    f = {k: np.asarray(v, dtype=np.float32) for k, v in inputs.items()}
    s0, s1 = float(np.exp(f["log_scale"][0])), float(np.exp(f["log_scale"][1]))
    w1 = f["mlp_w1"]
    w1c0 = (w1 * s0).copy()
    w1c1 = (w1 * s1).copy()
    w1c0[0, :] /= EPSX
    w1c1[0, :] /= EPSX
    w1big = np.zeros((128, 128), np.float32)
    for c in range(4):
        w1big[32 * c:32 * c + 9, 0:64] = w1c0
        w1big[32 * c + 9:32 * c + 18, 64:128] = w1c1

    sc18 = np.array([EPSX, 1, 1, 2, 2, 4, 4, 8, 8] * 2, np.float32)
    sb18 = np.array([0] + [0, PI / 2] * 4, np.float32)
    sb18 = np.concatenate([sb18, sb18])
    sinscale = np.ones((128, 1), np.float32)
    sinbias = np.zeros((128, 1), np.float32)
    for c in range(4):
        sinscale[32 * c:32 * c + 18, 0] = sc18
        sinbias[32 * c:32 * c + 18, 0] = sb18

    b1s = np.concatenate([f["mlp_b1"], f["mlp_b1"]]).reshape(128, 1)

    g0, g1 = f["film_gamma"][0], f["film_gamma"][1]
    be0, be1 = f["film_beta"][0], f["film_beta"][1]
    b2 = f["mlp_b2"]
    b2e = np.concatenate([g0 * b2 + be0, g1 * b2 + be1])
    if np.any(b2e != 0):
        raise NotImplementedError("nonzero folded e-bias not supported")
    W2blk = np.zeros((128, 128), np.float32)
    W2blk[0:64, 0:64] = f["mlp_w2"] * g0[None, :]
    W2blk[64:128, 64:128] = f["mlp_w2"] * g1[None, :]

    glW = (W2blk @ f["gate_w1"]).astype(np.float32)          # (128, 64)
    glWE = np.zeros((128, 128), np.float32); glWE[:, 0:64] = glW
    glWO = np.zeros((128, 128), np.float32); glWO[:, 64:128] = glW
    gb1 = f["gate_b1"].reshape(64, 1)
    gb1s = np.concatenate([gb1, gb1]).reshape(128, 1)
    temp = float(np.exp(f["gate_temp"]))
    gw2d = (f["gate_w2"][:, 0] - f["gate_w2"][:, 1]) / temp
    gb2d = float((f["gate_b2"][0] - f["gate_b2"][1]) / temp)
    gp = f["ln_g"] * f["proj_w"][:, 0]
    gps64 = float(gp.sum() / 64.0)
    if float(f["proj_b"][0]) != 0.0 or np.any(f["ln_b"] != 0):
        # ln_b contributes ln_b @ proj_w (a constant) and proj_b adds directly;
        # both shift adapt_bias by a constant -> softmax-invariant. Safe to drop.
        pass

    ones64 = np.ones(64, np.float32)
    vA0 = W2blk[:, 0:64] @ ones64
    vA1 = W2blk[:, 64:128] @ ones64
    vB0 = W2blk[:, 0:64] @ gp
    vB1 = W2blk[:, 64:128] @ gp
    # 6 blocks of [128,32], stats in cols 0..7; chunk c's band is psum
    # partitions 32c..32c+32 (4 concurrent 128x32 col-tiles on the PE).
    statsW = np.zeros((128, 192), np.float32)
    statsW[:, 0] = vA0 - vA1                           # S0 = a0 - a1
    statsW[:, 1] = vA1                                 # S1 = a1
    statsW[:, 2] = vB0 - vB1                           # S2 = b0 - b1
    statsW[:, 3] = vB1                                 # S3 = b1
    statsW[:, 32 + 4] = 1.0                            # q00 + q11 (toward S4)
    statsW[64:128, 32 + 5] = 1.0                       # S5 = q11
    statsW[64:128, 32 + 6] = -1.0                      # -q11 (toward S6)
    statsW[0:64, 64 + 4] = -2.0                        # -2qc (crx even half)
    statsW[0:64, 64 + 6] = 1.0                         # +qc
    statsW[64:128, 96 + 4] = -2.0                      # -2qc (crx odd half)
    statsW[64:128, 96 + 6] = 1.0                       # +qc
    statsW[0:64, 128 + 7] = gw2d                       # d (gg even half)
    statsW[64:128, 160 + 7] = gw2d                     # d (gg odd half)

    selT = np.zeros((128, 32), np.float32)
    for c in range(4):
        for s in range(8):
            selT[32 * c + s, 8 * c + s] = 1.0

    selC = np.zeros((128, 4096), np.float32)
    selA = np.zeros((128, 4096), np.float32)
    for q in range(NQ):
        for c in range(4):
            i = 4 * q + c
            for r in range(9):
                selC[i, 128 * q + 32 * c + r] = 1.0
                selA[i, 128 * q + 32 * c + 9 + r] = 1.0

    packF = np.zeros((128, 1033), np.float32)
    packF[:, 0] = sinscale[:, 0]
    packF[:, 1] = sinbias[:, 0]
    packF[:, 2] = b1s[:, 0]
    packF[:, 3] = gb1s[:, 0]
    packF[:, 4] = f["bq"]
    packF[:, 5:9] = f["ff_b1"].reshape(4, 128).T
    packF[:, 9:137] = np.eye(128, dtype=np.float32)
    packF[:, 137:265] = 1.0                              # onesB
    packF[:, 265:393] = np.broadcast_to(f["bv"], (128, H))
    # 393:521 = xrows (per-core, filled below)
    packF[:, 521:649] = np.broadcast_to(f["bo"], (RPC, D))
    packF[:, 649:777] = np.broadcast_to(f["rms1"], (RPC, D))
    packF[:, 777:905] = np.broadcast_to(f["rms2"], (RPC, D))
    packF[:, 905:1033] = np.broadcast_to(f["ff_b2"], (RPC, D))

    packB = np.zeros((128, 3456), np.float32)
    # 0:512 xT, 512:640 xrowsT, 640:1152 cost (per-core, filled below)
    packB[:, 1152:1280] = w1big
    packB[:, 1280:1408] = W2blk
    packB[:, 1408:1536] = glWE
    packB[:, 1536:1664] = glWO
    packB[:, 1664:1856] = statsW
    packB[:, 1856:1888] = selT
    packB[:, 1888:1920] = 1.0                            # onesM
    packB[:, 1920:2048] = f["wq"]
    packB[:, 2048:2176] = f["wk"]
    packB[:, 2176:2304] = f["wv"]
    packB[:, 2304:2432] = f["wo"]
    packB[:, 2432:2944] = f["ff_w1"]
    packB[:, 2944:3456] = np.concatenate(
        [f["ff_w2"][128 * fb:128 * fb + 128, :] for fb in range(4)], axis=1)

    shared = {
        "selC": selC.astype(BF), "selA": selA.astype(BF),
    }

    in_maps = []
    for core in range(NCORES):
        b = core // 4
        r0 = (core % 4) * RPC
        xb = f["x"][b]
        cx = f["coords"][b, :, 0]
        cy = f["coords"][b, :, 1]
        pF = packF.copy()
        pF[:, 393:521] = xb[r0:r0 + RPC] + f["bo"][None, :]
        pB = packB.copy()
        pB[:, 0:512] = xb.T
        pB[:, 512:640] = xb[r0:r0 + RPC].T
        pB[:, 640:1152] = f["cost_mat"][b, r0:r0 + RPC]
        p2 = np.zeros((2, 1280), np.float32)
        p2[:, 0:RPC] = np.stack([cx[r0:r0 + RPC], np.ones(RPC, np.float32)])
        p2[:, RPC:2 * RPC] = np.stack([cy[r0:r0 + RPC], np.ones(RPC, np.float32)])
        p2[:, 256:256 + T] = np.stack([np.ones(T, np.float32), -cx])
        p2[:, 768:768 + T] = np.stack([np.ones(T, np.float32), -cy])
        m = dict(shared)
        m.update({
            "pack2": p2,
            "packF": pF,
            "packB": pB.astype(BF),
        })
        in_maps.append(m)
    return in_maps, {"gb2d": gb2d, "gps64": gps64}


def _get_program(consts):
    key = tuple(sorted(consts.items()))
    if key not in _CACHE:
        nc, loc = _build_program()
        tt = {k: v for k, v in loc.items() if k.startswith("t_")}
        nc = _emit(nc, tt, consts)
        _CACHE[key] = nc
    return _CACHE[key]


def kernel(**inputs):
    in_maps, consts = _prepare(inputs)
    nc = _get_program(consts)
    res = bass_utils.run_bass_kernel_spmd(nc, in_maps, core_ids=list(range(NCORES)))
    out = np.zeros((B, T, D), np.float32)
    for core in range(NCORES):
        b = core // 4
        r0 = (core % 4) * RPC
        out[b, r0:r0 + RPC] = res.results[core]["out"]
    return out

